# revision 7
# baseline (speedup 1.0000x reference)
"""Multi-head self-attention (B=2, T=2048, d_model=1024, 16 heads, causal)
on 8 trn2 NeuronCores.

Sharding: core c -> batch b=c//4, head-group g=c%4 (4 heads, d_model slice
of 256). Each core computes its heads' attention and a partial wo
projection [2048, 1024] (bf16); host sums the 4 partials per batch and
adds bo + bv @ wo^T (the V-bias commutes through softmax since rows sum
to 1, so it is a host-side constant).

Per-core pipeline (all matmul inputs bf16, fp32 PSUM accumulation):
  QT[dq,t] = (wq_s @ x^T)*0.125 + bq*0.125   (scale folded into weights)
  KT[dk,t] = wk_s @ x^T + bk
  V[t,dv]  = x @ wv_s^T
  per head pair, per q-block j (512 wide):
    S^T[k,q] = K_h @ Q_h^T   (K=64 contraction, heads packed at partition
                              bases 0/64 -> concurrent row-group matmuls)
    E = exp(S^T)             (ACT, one call per (head-pair, ktile))
    diag tiles: E *= upper-tri mask
    O^T_aug = V_aug^T @ E    (V_aug = [64 ones cols | V_h cols] so PSUM rows
                              0-63 = replicated rowsums, rows 64-127 = O^T)
    OT_all = O^T * recip(rowsum)  (DVE reciprocal + tensor_tensor mul)
  P = OT_all^T @ wo_s^T      (partial output, bf16 out)

Scheduling: the per-ktile chain S(PE) -> exp(ACT) -> PV(PE) leaves PE
under-filled during attention (ACT needs ~1.15us per ktile vs ~0.64us of
attention PE work).  Projection and wo matmuls for neighboring q-blocks
are emitted as generator-based filler chunks (~0.4us of PE work per
chunk) interleaved between attention ktile steps; wo work is held back
to round 3, the most ACT-bound stretch.  x is staged t-slice-major
(both DRAM and SBUF) so each input DMA moves 8KB-contiguous lines per
partition at full rate and projections of t-slice 0 start ~10us in.
"""
import sys
sys.path.insert(0, "/opt/trn_rl_repo")
from collections import deque

import numpy as np
import ml_dtypes

import concourse.bass as bass
import concourse.bacc as bacc
import concourse.tile as tile
import concourse.mybir as mybir
from concourse import bass_utils

BF16 = mybir.dt.bfloat16
F32 = mybir.dt.float32
EXP = mybir.ActivationFunctionType.Exp

T = 2048          # sequence length
DM = 1024         # d_model
DS = 256          # per-core d_model slice (4 heads x 64)
HD = 64           # head dim
NH = 4            # heads per core
KT128 = 16        # k tiles of 128 over T
QB = 512          # q block width
NJ = T // QB      # 4 q blocks
NCORES = 8

_CACHE = {}


def _build():
    nc = bacc.Bacc("TRN2", target_bir_lowering=False, debug=False,
                   enable_asserts=False, num_devices=NCORES)
    dram = {}
    for name, shape, dt in [
        ("xt", [NJ, 128, 8, 512], BF16),  # x[b]^T, t-slice-major chunks
        ("wqt", [DM, DS], BF16),      # wq.T[:, slice] * 0.125
        ("wkt", [DM, DS], BF16),
        ("wvt", [DM, DS], BF16),
        ("wot", [DS, DM], BF16),      # wo[:, slice].T
        ("bqc", [128, 2], F32),       # bq*0.125 as [128, m] columns
        ("bkc", [128, 2], F32),
        ("tri", [128, 128], BF16),    # upper-tri (incl diag) ones
    ]:
        dram[name] = nc.dram_tensor(name, shape, dt, kind="ExternalInput").ap()
    p_out = nc.dram_tensor("p_out", [T, DM], BF16, kind="ExternalOutput").ap()

    with tile.TileContext(nc) as tc:
        with tc.tile_pool(name="persist", bufs=1) as pp, \
             tc.tile_pool(name="epool", bufs=3) as ep, \
             tc.tile_pool(name="outp", bufs=2) as op, \
             tc.tile_pool(name="bcp", bufs=2) as bp, \
             tc.tile_pool(name="misc_ps", bufs=2, space="PSUM") as mp, \
             tc.tile_pool(name="st_ps", bufs=2, space="PSUM") as sp, \
             tc.tile_pool(name="ot_ps", bufs=1, space="PSUM") as tp:

            # ---- persistent SBUF ----
            xt = pp.tile([128, NJ, 8, 512], BF16, name="xt")  # [p, ts, kt8, c]
            wqt = pp.tile([128, 8, DS], BF16, name="wqt")
            wkt = pp.tile([128, 8, DS], BF16, name="wkt")
            wvt = pp.tile([128, 8, DS], BF16, name="wvt")
            wot = pp.tile([128, 2, DM], BF16, name="wot")
            qt = pp.tile([128, 2, T], BF16, name="qt")        # [p, dq-tile, t]
            kt = pp.tile([128, 2, T], BF16, name="kt")
            vaug = pp.tile([128, KT128, 512], BF16, name="vaug")
            ot_all = pp.tile([128, 2, T], BF16, name="ot_all")
            ones_row = pp.tile([1, 512], BF16, name="ones_row")
            bqc = pp.tile([128, 2], F32, name="bqc")
            bkc = pp.tile([128, 2], F32, name="bkc")
            tri = pp.tile([128, 128], BF16, name="tri")

            # ones_row feeds the PE warmup; DVE is otherwise idle at start.
            nc.vector.memset(ones_row, 1.0)
            # PE warmup: dummy matmuls on scratch during the input DMA wait so
            # HAM is at full clock when real matmuls start (results unread)
            warm = mp.tile([128, 512], F32, name="warm", tag="mp")
            for _ in range(9):
                nc.tensor.matmul(warm, lhsT=ones_row[0:1, 0:128],
                                 rhs=ones_row[0:1, :], start=True, stop=True)

            # Input DMA. A single queue sustains only ~100GB/s, so the head
            # load is split in priority order across the three trigger-capable
            # engines (sync/scalar/gpsimd), in chunks small enough that the
            # proj(0) contraction can start while the rest streams in.
            def w_dram(nm):
                return dram[nm].rearrange("(kt p) d -> p kt d", p=128)

            # x t-slice 0 per-k chunks, round-robin across queues
            x0q = [nc.sync, nc.scalar, nc.gpsimd]
            for k in range(8):
                x0q[k % 3].dma_start(out=xt[:, 0, k], in_=dram["xt"][0, :, k])
            wq_d, wk_d, wv_d, wo_d = (w_dram(n) for n in
                                      ("wqt", "wkt", "wvt", "wot"))
            # m0 halves of wq/wk gate the first projections
            nc.gpsimd.dma_start(out=wqt[:, :, 0:128], in_=wq_d[:, :, 0:128])
            nc.scalar.dma_start(out=wkt[:, :, 0:128], in_=wk_d[:, :, 0:128])
            nc.sync.dma_start(out=wvt, in_=wv_d)
            nc.gpsimd.dma_start(out=wqt[:, :, 128:256], in_=wq_d[:, :, 128:256])
            nc.scalar.dma_start(out=wkt[:, :, 128:256], in_=wk_d[:, :, 128:256])
            nc.scalar.dma_start(out=bqc, in_=dram["bqc"])
            nc.scalar.dma_start(out=bkc, in_=dram["bkc"])
            nc.scalar.dma_start(out=tri, in_=dram["tri"])
            # later t-slices, one per queue so each lands before its round
            nc.sync.dma_start(out=xt[:, 1], in_=dram["xt"][1])
            nc.scalar.dma_start(out=xt[:, 2], in_=dram["xt"][2])
            nc.gpsimd.dma_start(out=wot, in_=wo_d)
            nc.gpsimd.dma_start(out=xt[:, 3], in_=dram["xt"][3])
            # V_aug head block h: cols [128h, 128h+64) ones, [128h+64, +128) V.
            # After the DMA triggers so they don't delay the weight loads.
            for h in range(NH):
                nc.gpsimd.memset(vaug[:, :, 128 * h:128 * h + HD], 1.0)

            # ---- emission helpers ----
            def proj_qk_m(w_sb, b_c, dst, ts, m, chunk=99):
                """One [128,512] psum tile of the Q/K projection; yields every
                `chunk` contraction matmuls."""
                t0 = ts * 512
                ps = mp.tile([128, 512], F32, name="proj_ps", tag="mp")
                for k in range(8):
                    nc.tensor.matmul(
                        ps, lhsT=w_sb[:, k, m * 128:(m + 1) * 128],
                        rhs=xt[:, ts, k, :],
                        start=(k == 0), stop=(k == 7))
                    if k % chunk == chunk - 1 and k < 7:
                        yield
                nc.vector.tensor_scalar_add(
                    dst[:, m, t0:t0 + 512], ps, b_c[:, m:m + 1])
                yield

            def proj_v_tt(g):
                """V projection for t-subtile g (128 rows): 8 matmuls + the
                scatter into vaug."""
                ps = mp.tile([128, 256], F32, name="v_ps", tag="mp")
                for k in range(8):
                    nc.tensor.matmul(
                        ps, lhsT=xt[:, g // 4, k, (g % 4) * 128:(g % 4 + 1) * 128],
                        rhs=wvt[:, k, :], start=(k == 0), stop=(k == 7))
                    if k == 3:
                        yield
                # scatter into vaug: head h -> cols [128h+64, 128h+128)
                nc.vector.tensor_copy(
                    vaug[:, g, :].rearrange("p (h c) -> p h c", h=NH)[:, :, HD:],
                    ps.rearrange("p (h c) -> p h c", h=NH))
                yield

            def gen_proj_qk(ts):
                for w_sb, b_c, dst in ((wqt, bqc, qt), (wkt, bkc, kt)):
                    for m in range(2):
                        yield from proj_qk_m(w_sb, b_c, dst, ts, m, chunk=2)

            def gen_proj_v(ts):
                for tt in range(4):
                    yield from proj_v_tt(4 * ts + tt)

            def gen_wo(j):
                q0 = j * QB
                last = j == NJ - 1
                ob = op.tile([128, 4, DM], BF16, name="ob", tag="ob")
                for qq in range(4):
                    row = q0 + qq * 128
                    for n in range(2):
                        ps = mp.tile([128, 512], F32, name="wo_ps", tag="mp")
                        for kk in range(2):
                            nc.tensor.matmul(
                                ps, lhsT=ot_all[:, kk, row:row + 128],
                                rhs=wot[:, kk, n * 512:(n + 1) * 512],
                                start=(kk == 0), stop=(kk == 1))
                        nc.vector.tensor_copy(ob[:, qq, n * 512:(n + 1) * 512],
                                              ps)
                        yield
                    if last:
                        # tail round: stream the output per 128-row chunk so
                        # the final DMA only trails the last copy
                        dma_eng = nc.sync if qq % 2 == 0 else nc.gpsimd
                        dma_eng.dma_start(
                            out=p_out[row:row + 128, :], in_=ob[:, qq, :])
                if not last:
                    dma_eng = nc.sync if j % 2 == 0 else nc.gpsimd
                    dma_eng.dma_start(
                        out=p_out[q0:q0 + 512, :].rearrange(
                            "(q p) d -> p q d", p=128),
                        in_=ob)
                yield

            # Global filler queue: (key, generator), drained strictly FIFO.
            fillers = deque()

            def fill(n):
                while n > 0 and fillers:
                    try:
                        next(fillers[0][1])
                        n -= 1
                    except StopIteration:
                        fillers.popleft()

            def force(key):
                """Drain fillers until generator `key` has been exhausted."""
                while any(k == key for k, _ in fillers):
                    try:
                        next(fillers[0][1])
                    except StopIteration:
                        fillers.popleft()

            def attention(j, fill_fn):
                q0 = j * QB
                nk = 4 * (j + 1)           # k-tiles of 128
                for H in range(2):          # head pair (2H, 2H+1)
                    ot = [tp.tile([128, QB], F32, name=f"ot{hp}", tag=f"ot{hp}")
                          for hp in range(2)]
                    for ktile in range(nk):
                        s = ktile - 4 * j       # >=0 on diag block
                        c0 = 128 * s if s >= 0 else 0
                        st = sp.tile([128, 2, 512], F32, name="st", tag="st")
                        e_t = ep.tile([128, 2, QB], BF16, name="e", tag="e")
                        for hp in range(2):
                            h = 2 * H + hp
                            r0 = (HD * h) % 128
                            mi = (HD * h) // 128
                            nc.tensor.matmul(
                                st[:, hp, c0:512],
                                lhsT=kt[r0:r0 + HD, mi,
                                        ktile * 128:(ktile + 1) * 128],
                                rhs=qt[r0:r0 + HD, mi, q0 + c0:q0 + QB],
                                start=True, stop=True)
                        nc.scalar.activation(
                            out=e_t, in_=st, func=EXP, scale=1.0)
                        if s >= 0:
                            # mask both heads' diag tile in one strided op
                            dg = e_t[:, :, c0:c0 + 128]
                            trb = bass.AP(
                                tensor=tri.tensor, offset=tri.offset,
                                ap=[tri.ap[0], [0, 2], [1, 128]])
                            nc.vector.tensor_mul(dg, dg, trb)
                        fill_fn(j, H, ktile)
                        for hp in range(2):
                            h = 2 * H + hp
                            nc.tensor.matmul(
                                ot[hp][:, c0:QB],
                                lhsT=vaug[:, ktile, 128 * h:128 * (h + 1)],
                                rhs=e_t[:, hp, c0:QB],
                                start=(ktile == 0), stop=(ktile == nk - 1))
                    for hp in range(2):
                        h = 2 * H + hp
                        rec = bp.tile([64, QB], F32, name="rec", tag="rec")
                        nc.vector.reciprocal_approx_fast(rec, ot[hp][0:64, :])
                        r0 = (HD * h) % 128
                        mi = (HD * h) // 128
                        nc.vector.tensor_mul(
                            ot_all[r0:r0 + HD, mi, q0:q0 + QB],
                            ot[hp][64:128, :], rec)

            # ---- schedule ----
            # Round 0 prefix: Q m0, K m0, V g=0 -> attention(0) H0 can start;
            # the rest of proj(0) rides along as round-0 filler.
            for _ in proj_qk_m(wqt, bqc, qt, 0, 0):
                pass
            for _ in proj_qk_m(wkt, bkc, kt, 0, 0):
                pass
            for _ in proj_v_tt(0):
                pass
            local = deque()
            for g in (proj_v_tt(1), proj_v_tt(2), proj_v_tt(3),
                      proj_qk_m(wqt, bqc, qt, 0, 1, chunk=4),
                      proj_qk_m(wkt, bkc, kt, 0, 1, chunk=4)):
                local.append(("r0", g))

            def fill_r0(j, H, ktile):
                n = 3
                while n > 0 and local:
                    try:
                        next(local[0][1])
                        n -= 1
                    except StopIteration:
                        local.popleft()
                if not local:
                    fill(1)

            for ts in range(1, NJ):
                fillers.append((("qk", ts), gen_proj_qk(ts)))
                fillers.append((("v", ts), gen_proj_v(ts)))

            def fill_main(j, H, ktile):
                if H == 0 and ktile == 4 * j:
                    force(("v", j))   # vaug t-slice j gate (usually a no-op)
                fill(1)

            attention(0, fill_r0)
            while local:
                fill_r0(0, 0, 0)
            for j in range(1, NJ):
                force(("qk", j))
                if j == NJ - 1:
                    # wo is the only filler left whose deadline is the kernel
                    # end; spend it on the most ACT-bound round.
                    for jj in range(NJ - 1):
                        fillers.append((("wo", jj), gen_wo(jj)))
                attention(j, fill_main)
            while fillers:
                fill(64)
            for _ in gen_wo(NJ - 1):
                pass
    nc.compile()
    return nc


def _prep_inputs(x, wq, bq, wk, bk, wv, wo):
    bf = ml_dtypes.bfloat16
    scale = np.float32(1.0 / np.sqrt(HD))
    tri = np.triu(np.ones((128, 128), np.float32)).astype(bf)
    in_maps = []
    for c in range(NCORES):
        b, g = c // 4, c % 4
        sl = slice(DS * g, DS * (g + 1))
        xts = np.ascontiguousarray(
            x[b].T.astype(bf).reshape(8, 128, 4, 512).transpose(2, 1, 0, 3))
        in_maps.append({
            "xt": xts,
            "wqt": np.ascontiguousarray(wq.T[:, sl] * scale).astype(bf),
            "wkt": np.ascontiguousarray(wk.T[:, sl]).astype(bf),
            "wvt": np.ascontiguousarray(wv.T[:, sl]).astype(bf),
            "wot": np.ascontiguousarray(wo[:, sl].T).astype(bf),
            "bqc": np.ascontiguousarray(
                (bq[sl] * scale).reshape(2, 128).T).astype(np.float32),
            "bkc": np.ascontiguousarray(
                bk[sl].reshape(2, 128).T).astype(np.float32),
            "tri": tri,
        })
    return in_maps


TRACE = False
TRACE_DIR = None
LAST_RESULT = None


def kernel(x, wq, bq, wk, bk, wv, bv, wo, bo):
    global LAST_RESULT
    x, wq, bq, wk, bk, wv, bv, wo, bo = [
        np.asarray(a, np.float32)
        for a in (x, wq, bq, wk, bk, wv, bv, wo, bo)]
    if "nc" not in _CACHE:
        _CACHE["nc"] = _build()
    nc = _CACHE["nc"]
    in_maps = _prep_inputs(x, wq, bq, wk, bk, wv, wo)
    res = bass_utils.run_bass_kernel_spmd(
        nc, in_maps, core_ids=list(range(NCORES)), trace=TRACE,
        tmpdir=TRACE_DIR)
    LAST_RESULT = res
    # bv commutes through softmax (rows sum to 1): out += bv @ wo^T + bo.
    const_row = (bv.astype(np.float64) @ wo.T.astype(np.float64) +
                 bo.astype(np.float64)).astype(np.float32)
    out = np.empty((2, T, DM), np.float32)
    for b in range(2):
        acc = res.results[4 * b]["p_out"].astype(np.float32)
        for g in range(1, 4):
            acc = acc + res.results[4 * b + g]["p_out"].astype(np.float32)
        out[b] = acc + const_row
    return out


# revision 9
# speedup vs baseline: 1.0229x; 1.0229x over previous
"""Multi-head self-attention (B=2, T=2048, d_model=1024, 16 heads, causal)
on 8 trn2 NeuronCores.

Sharding: core c -> batch b=c//4, head-group g=c%4 (4 heads, d_model slice
of 256). Each core computes its heads' attention and a partial wo
projection [2048, 1024] (bf16); host sums the 4 partials per batch and
adds bo + bv @ wo^T (the V-bias commutes through softmax since rows sum
to 1, so it is a host-side constant).

Per-core pipeline (all matmul inputs bf16, fp32 PSUM accumulation):
  QT[dq,t] = (wq_s @ x^T)*0.125 + bq*0.125   (scale folded into weights)
  KT[dk,t] = wk_s @ x^T + bk
  V[t,dv]  = x @ wv_s^T
  per head pair, per q-block j (512 wide):
    S^T[k,q] = K_h @ Q_h^T   (K=64 contraction, heads packed at partition
                              bases 0/64 -> concurrent row-group matmuls)
    E = exp(S^T)             (ACT, one call per (head-pair, ktile))
    diag tiles: E *= upper-tri mask
    O^T_aug = V_aug^T @ E    (V_aug = [64 ones cols | V_h cols] so PSUM rows
                              0-63 = replicated rowsums, rows 64-127 = O^T)
    OT_all = O^T * recip(rowsum)  (DVE reciprocal + tensor_tensor mul)
  P = OT_all^T @ wo_s^T      (partial output, bf16 out)

Scheduling: the per-ktile chain S(PE) -> exp(ACT) -> PV(PE) leaves PE
under-filled during attention (ACT needs ~1.15us per ktile vs ~0.64us of
attention PE work).  Projection and wo matmuls for neighboring q-blocks
are emitted as generator-based filler chunks (~0.4us of PE work per
chunk) interleaved between attention ktile steps; wo work is held back
to round 3, the most ACT-bound stretch.  x is staged t-slice-major
(both DRAM and SBUF) so each input DMA moves 8KB-contiguous lines per
partition at full rate and projections of t-slice 0 start ~10us in.
"""
import sys
sys.path.insert(0, "/opt/trn_rl_repo")
from collections import deque

import numpy as np
import ml_dtypes

import concourse.bass as bass
import concourse.bacc as bacc
import concourse.tile as tile
import concourse.mybir as mybir
from concourse import bass_utils

BF16 = mybir.dt.bfloat16
F32 = mybir.dt.float32
EXP = mybir.ActivationFunctionType.Exp

T = 2048          # sequence length
DM = 1024         # d_model
DS = 256          # per-core d_model slice (4 heads x 64)
HD = 64           # head dim
NH = 4            # heads per core
KT128 = 16        # k tiles of 128 over T
QB = 512          # q block width
NJ = T // QB      # 4 q blocks
NCORES = 8

_CACHE = {}


def _build():
    nc = bacc.Bacc("TRN2", target_bir_lowering=False, debug=False,
                   enable_asserts=False, num_devices=NCORES)
    dram = {}
    for name, shape, dt in [
        ("xt", [NJ, 128, 8, 512], BF16),  # x[b]^T, t-slice-major chunks
        ("wqt", [DM, DS], BF16),      # wq.T[:, slice] * 0.125
        ("wkt", [DM, DS], BF16),
        ("wvt", [DM, DS], BF16),
        ("wot", [DS, DM], BF16),      # wo[:, slice].T
        ("bqc", [128, 2], F32),       # bq*0.125 as [128, m] columns
        ("bkc", [128, 2], F32),
        ("tri", [128, 128], BF16),    # upper-tri (incl diag) ones
    ]:
        dram[name] = nc.dram_tensor(name, shape, dt, kind="ExternalInput").ap()
    p_out = nc.dram_tensor("p_out", [T, DM], BF16, kind="ExternalOutput").ap()

    with tile.TileContext(nc) as tc:
        with tc.tile_pool(name="persist", bufs=1) as pp, \
             tc.tile_pool(name="epool", bufs=3) as ep, \
             tc.tile_pool(name="outp", bufs=2) as op, \
             tc.tile_pool(name="bcp", bufs=2) as bp, \
             tc.tile_pool(name="misc_ps", bufs=2, space="PSUM") as mp, \
             tc.tile_pool(name="st_ps", bufs=2, space="PSUM") as sp, \
             tc.tile_pool(name="ot_ps", bufs=1, space="PSUM") as tp:

            # ---- persistent SBUF ----
            xt = pp.tile([128, NJ, 8, 512], BF16, name="xt")  # [p, ts, kt8, c]
            wqt = pp.tile([128, 8, DS], BF16, name="wqt")
            wkt = pp.tile([128, 8, DS], BF16, name="wkt")
            wvt = pp.tile([128, 8, DS], BF16, name="wvt")
            wot = pp.tile([128, 2, DM], BF16, name="wot")
            qt = pp.tile([128, 2, T], BF16, name="qt")        # [p, dq-tile, t]
            kt = pp.tile([128, 2, T], BF16, name="kt")
            vaug = pp.tile([128, KT128, 512], BF16, name="vaug")
            ot_all = pp.tile([128, 2, T], BF16, name="ot_all")
            ones_row = pp.tile([1, 512], BF16, name="ones_row")
            bqc = pp.tile([128, 2], F32, name="bqc")
            bkc = pp.tile([128, 2], F32, name="bkc")
            tri = pp.tile([128, 128], BF16, name="tri")

            # ones_row feeds the PE warmup; DVE is otherwise idle at start.
            nc.vector.memset(ones_row, 1.0)
            # PE warmup: dummy matmuls on scratch during the input DMA wait so
            # HAM is at full clock when real matmuls start (results unread)
            warm = mp.tile([128, 512], F32, name="warm", tag="mp")
            for _ in range(9):
                nc.tensor.matmul(warm, lhsT=ones_row[0:1, 0:128],
                                 rhs=ones_row[0:1, :], start=True, stop=True)

            # Input DMA. A single queue sustains only ~100GB/s, so the head
            # load is split in priority order across the three trigger-capable
            # engines (sync/scalar/gpsimd), in chunks small enough that the
            # proj(0) contraction can start while the rest streams in.
            def w_dram(nm):
                return dram[nm].rearrange("(kt p) d -> p kt d", p=128)

            wq_d, wk_d, wv_d, wo_d = (w_dram(n) for n in
                                      ("wqt", "wkt", "wvt", "wot"))
            # ~2.5MB gates attention(0); balance it across the three queues
            # in strict need-order (x0 + m0 weight halves first).
            nc.sync.dma_start(out=xt[:, 0, 0:3], in_=dram["xt"][0, :, 0:3])
            nc.sync.dma_start(out=wqt[:, :, 0:128], in_=wq_d[:, :, 0:128])
            nc.sync.dma_start(out=xt[:, 1], in_=dram["xt"][1])
            nc.gpsimd.dma_start(out=xt[:, 0, 3:6], in_=dram["xt"][0, :, 3:6])
            nc.gpsimd.dma_start(out=wkt[:, :, 0:128], in_=wk_d[:, :, 0:128])
            nc.gpsimd.dma_start(out=wvt, in_=wv_d)
            nc.gpsimd.dma_start(out=wot, in_=wo_d)
            nc.gpsimd.dma_start(out=xt[:, 3], in_=dram["xt"][3])
            nc.scalar.dma_start(out=bqc, in_=dram["bqc"])
            nc.scalar.dma_start(out=bkc, in_=dram["bkc"])
            nc.scalar.dma_start(out=tri, in_=dram["tri"])
            nc.scalar.dma_start(out=xt[:, 0, 6:8], in_=dram["xt"][0, :, 6:8])
            nc.scalar.dma_start(out=wqt[:, :, 128:256], in_=wq_d[:, :, 128:256])
            nc.scalar.dma_start(out=wkt[:, :, 128:256], in_=wk_d[:, :, 128:256])
            nc.scalar.dma_start(out=xt[:, 2], in_=dram["xt"][2])
            # V_aug head block h: cols [128h, 128h+64) ones, [128h+64, +128) V.
            # After the DMA triggers so they don't delay the weight loads.
            for h in range(NH):
                nc.gpsimd.memset(vaug[:, :, 128 * h:128 * h + HD], 1.0)

            # ---- emission helpers ----
            def proj_qk_m(w_sb, b_c, dst, ts, m, chunk=99):
                """One [128,512] psum tile of the Q/K projection; yields every
                `chunk` contraction matmuls."""
                t0 = ts * 512
                ps = mp.tile([128, 512], F32, name="proj_ps", tag="mp")
                for k in range(8):
                    nc.tensor.matmul(
                        ps, lhsT=w_sb[:, k, m * 128:(m + 1) * 128],
                        rhs=xt[:, ts, k, :],
                        start=(k == 0), stop=(k == 7))
                    if k % chunk == chunk - 1 and k < 7:
                        yield
                nc.vector.tensor_scalar_add(
                    dst[:, m, t0:t0 + 512], ps, b_c[:, m:m + 1])
                yield

            def proj_v_tt(g):
                """V projection for t-subtile g (128 rows): 8 matmuls + the
                scatter into vaug."""
                ps = mp.tile([128, 256], F32, name="v_ps", tag="mp")
                for k in range(8):
                    nc.tensor.matmul(
                        ps, lhsT=xt[:, g // 4, k, (g % 4) * 128:(g % 4 + 1) * 128],
                        rhs=wvt[:, k, :], start=(k == 0), stop=(k == 7))
                    if k == 3:
                        yield
                # scatter into vaug: head h -> cols [128h+64, 128h+128)
                nc.vector.tensor_copy(
                    vaug[:, g, :].rearrange("p (h c) -> p h c", h=NH)[:, :, HD:],
                    ps.rearrange("p (h c) -> p h c", h=NH))
                yield

            def gen_proj_qk(ts):
                for w_sb, b_c, dst in ((wqt, bqc, qt), (wkt, bkc, kt)):
                    for m in range(2):
                        yield from proj_qk_m(w_sb, b_c, dst, ts, m, chunk=2)

            def gen_proj_v(ts):
                for tt in range(4):
                    yield from proj_v_tt(4 * ts + tt)

            def gen_wo(j):
                q0 = j * QB
                last = j == NJ - 1
                ob = op.tile([128, 4, DM], BF16, name="ob", tag="ob")
                for qq in range(4):
                    row = q0 + qq * 128
                    for n in range(2):
                        ps = mp.tile([128, 512], F32, name="wo_ps", tag="mp")
                        for kk in range(2):
                            nc.tensor.matmul(
                                ps, lhsT=ot_all[:, kk, row:row + 128],
                                rhs=wot[:, kk, n * 512:(n + 1) * 512],
                                start=(kk == 0), stop=(kk == 1))
                        nc.vector.tensor_copy(ob[:, qq, n * 512:(n + 1) * 512],
                                              ps)
                        yield
                    if last:
                        # tail round: stream the output per 128-row chunk so
                        # the final DMA only trails the last copy
                        dma_eng = nc.sync if qq % 2 == 0 else nc.gpsimd
                        dma_eng.dma_start(
                            out=p_out[row:row + 128, :], in_=ob[:, qq, :])
                if not last:
                    dma_eng = nc.sync if j % 2 == 0 else nc.gpsimd
                    dma_eng.dma_start(
                        out=p_out[q0:q0 + 512, :].rearrange(
                            "(q p) d -> p q d", p=128),
                        in_=ob)
                yield

            # Global filler queue: (key, generator), drained strictly FIFO.
            fillers = deque()

            def fill(n):
                while n > 0 and fillers:
                    try:
                        next(fillers[0][1])
                        n -= 1
                    except StopIteration:
                        fillers.popleft()

            def force(key):
                """Drain fillers until generator `key` has been exhausted."""
                while any(k == key for k, _ in fillers):
                    try:
                        next(fillers[0][1])
                    except StopIteration:
                        fillers.popleft()

            def attention(j, fill_fn):
                q0 = j * QB
                nk = 4 * (j + 1)           # k-tiles of 128
                for H in range(2):          # head pair (2H, 2H+1)
                    ot = [tp.tile([128, QB], F32, name=f"ot{hp}", tag=f"ot{hp}")
                          for hp in range(2)]
                    for ktile in range(nk):
                        s = ktile - 4 * j       # >=0 on diag block
                        c0 = 128 * s if s >= 0 else 0
                        st = sp.tile([128, 2, 512], F32, name="st", tag="st")
                        e_t = ep.tile([128, 2, QB], BF16, name="e", tag="e")
                        for hp in range(2):
                            h = 2 * H + hp
                            r0 = (HD * h) % 128
                            mi = (HD * h) // 128
                            nc.tensor.matmul(
                                st[:, hp, c0:512],
                                lhsT=kt[r0:r0 + HD, mi,
                                        ktile * 128:(ktile + 1) * 128],
                                rhs=qt[r0:r0 + HD, mi, q0 + c0:q0 + QB],
                                start=True, stop=True)
                        # diag tiles: only cols [c0:512) are computed/read --
                        # skip the stale region in the exp too
                        nc.scalar.activation(
                            out=e_t[:, :, c0:512], in_=st[:, :, c0:512],
                            func=EXP, scale=1.0)
                        if s >= 0:
                            # mask both heads' diag tile in one strided op
                            dg = e_t[:, :, c0:c0 + 128]
                            trb = bass.AP(
                                tensor=tri.tensor, offset=tri.offset,
                                ap=[tri.ap[0], [0, 2], [1, 128]])
                            nc.vector.tensor_mul(dg, dg, trb)
                        fill_fn(j, H, ktile)
                        for hp in range(2):
                            h = 2 * H + hp
                            nc.tensor.matmul(
                                ot[hp][:, c0:QB],
                                lhsT=vaug[:, ktile, 128 * h:128 * (h + 1)],
                                rhs=e_t[:, hp, c0:QB],
                                start=(ktile == 0), stop=(ktile == nk - 1))
                    for hp in range(2):
                        h = 2 * H + hp
                        rec = bp.tile([64, QB], F32, name="rec", tag="rec")
                        nc.vector.reciprocal_approx_fast(rec, ot[hp][0:64, :])
                        r0 = (HD * h) % 128
                        mi = (HD * h) // 128
                        nc.vector.tensor_mul(
                            ot_all[r0:r0 + HD, mi, q0:q0 + QB],
                            ot[hp][64:128, :], rec)

            # ---- schedule ----
            # Round 0 prefix: Q m0, K m0, V g=0 -> attention(0) H0 can start;
            # the rest of proj(0) rides along as round-0 filler.
            for _ in proj_qk_m(wqt, bqc, qt, 0, 0):
                pass
            for _ in proj_qk_m(wkt, bkc, kt, 0, 0):
                pass
            for _ in proj_v_tt(0):
                pass
            local = deque()
            for g in (proj_v_tt(1), proj_v_tt(2), proj_v_tt(3),
                      proj_qk_m(wqt, bqc, qt, 0, 1, chunk=4),
                      proj_qk_m(wkt, bkc, kt, 0, 1, chunk=4)):
                local.append(("r0", g))

            def fill_r0(j, H, ktile):
                n = 3
                while n > 0 and local:
                    try:
                        next(local[0][1])
                        n -= 1
                    except StopIteration:
                        local.popleft()
                if not local:
                    fill(1)

            for ts in range(1, NJ):
                fillers.append((("qk", ts), gen_proj_qk(ts)))
                fillers.append((("v", ts), gen_proj_v(ts)))

            def fill_main(j, H, ktile):
                if H == 0 and ktile == 4 * j:
                    force(("v", j))   # vaug t-slice j gate (usually a no-op)
                fill(1)

            attention(0, fill_r0)
            while local:
                fill_r0(0, 0, 0)
            for j in range(1, NJ):
                force(("qk", j))
                if j == NJ - 1:
                    # wo is the only filler left whose deadline is the kernel
                    # end; spend it on the most ACT-bound round.
                    for jj in range(NJ - 1):
                        fillers.append((("wo", jj), gen_wo(jj)))
                attention(j, fill_main)
            while fillers:
                fill(64)
            for _ in gen_wo(NJ - 1):
                pass
    nc.compile()
    return nc


def _prep_inputs(x, wq, bq, wk, bk, wv, wo):
    bf = ml_dtypes.bfloat16
    scale = np.float32(1.0 / np.sqrt(HD))
    tri = np.triu(np.ones((128, 128), np.float32)).astype(bf)
    in_maps = []
    for c in range(NCORES):
        b, g = c // 4, c % 4
        sl = slice(DS * g, DS * (g + 1))
        xts = np.ascontiguousarray(
            x[b].T.astype(bf).reshape(8, 128, 4, 512).transpose(2, 1, 0, 3))
        in_maps.append({
            "xt": xts,
            "wqt": np.ascontiguousarray(wq.T[:, sl] * scale).astype(bf),
            "wkt": np.ascontiguousarray(wk.T[:, sl]).astype(bf),
            "wvt": np.ascontiguousarray(wv.T[:, sl]).astype(bf),
            "wot": np.ascontiguousarray(wo[:, sl].T).astype(bf),
            "bqc": np.ascontiguousarray(
                (bq[sl] * scale).reshape(2, 128).T).astype(np.float32),
            "bkc": np.ascontiguousarray(
                bk[sl].reshape(2, 128).T).astype(np.float32),
            "tri": tri,
        })
    return in_maps


TRACE = False
TRACE_DIR = None
LAST_RESULT = None


def kernel(x, wq, bq, wk, bk, wv, bv, wo, bo):
    global LAST_RESULT
    x, wq, bq, wk, bk, wv, bv, wo, bo = [
        np.asarray(a, np.float32)
        for a in (x, wq, bq, wk, bk, wv, bv, wo, bo)]
    if "nc" not in _CACHE:
        _CACHE["nc"] = _build()
    nc = _CACHE["nc"]
    in_maps = _prep_inputs(x, wq, bq, wk, bk, wv, wo)
    res = bass_utils.run_bass_kernel_spmd(
        nc, in_maps, core_ids=list(range(NCORES)), trace=TRACE,
        tmpdir=TRACE_DIR)
    LAST_RESULT = res
    # bv commutes through softmax (rows sum to 1): out += bv @ wo^T + bo.
    const_row = (bv.astype(np.float64) @ wo.T.astype(np.float64) +
                 bo.astype(np.float64)).astype(np.float32)
    out = np.empty((2, T, DM), np.float32)
    for b in range(2):
        acc = res.results[4 * b]["p_out"].astype(np.float32)
        for g in range(1, 4):
            acc = acc + res.results[4 * b + g]["p_out"].astype(np.float32)
        out[b] = acc + const_row
    return out


# revision 12
# speedup vs baseline: 1.0499x; 1.0264x over previous
"""Multi-head self-attention (B=2, T=2048, d_model=1024, 16 heads, causal)
on 8 trn2 NeuronCores.

Sharding: core c -> batch b=c//4, head-group g=c%4 (4 heads, d_model slice
of 256). Each core computes its heads' attention and a partial wo
projection [2048, 1024] (bf16); host sums the 4 partials per batch and
adds bo + bv @ wo^T (the V-bias commutes through softmax since rows sum
to 1, so it is a host-side constant).

Per-core pipeline (all matmul inputs bf16, fp32 PSUM accumulation):
  QT[dq,t] = (wq_s @ x^T)*0.125 + bq*0.125   (scale folded into weights)
  KT[dk,t] = wk_s @ x^T + bk
  V[t,dv]  = x @ wv_s^T
  per head pair, per q-block j (512 wide):
    S^T[k,q] = K_h @ Q_h^T   (K=64 contraction, heads packed at partition
                              bases 0/64 -> concurrent row-group matmuls)
    E = exp(S^T)             (ACT, one call per (head-pair, ktile))
    diag tiles: E *= upper-tri mask
    O^T_aug = V_aug^T @ E    (V_aug = [64 ones cols | V_h cols] so PSUM rows
                              0-63 = replicated rowsums, rows 64-127 = O^T)
    OT_all = O^T * recip(rowsum)  (DVE reciprocal + tensor_tensor mul)
  P = OT_all^T @ wo_s^T      (partial output, bf16 out)

Scheduling: the per-ktile chain S(PE) -> exp(ACT) -> PV(PE) leaves PE
under-filled during attention (ACT needs ~1.15us per ktile vs ~0.64us of
attention PE work).  Projection and wo matmuls for neighboring q-blocks
are emitted as generator-based filler chunks (~0.4us of PE work per
chunk) interleaved between attention ktile steps; wo work is held back
to round 3, the most ACT-bound stretch.  x is staged t-slice-major
(both DRAM and SBUF) so each input DMA moves 8KB-contiguous lines per
partition at full rate and projections of t-slice 0 start ~10us in.
"""
import sys
sys.path.insert(0, "/opt/trn_rl_repo")
from collections import deque

import numpy as np
import ml_dtypes

import concourse.bass as bass
import concourse.bacc as bacc
import concourse.tile as tile
import concourse.mybir as mybir
from concourse import bass_utils

BF16 = mybir.dt.bfloat16
F32 = mybir.dt.float32
EXP = mybir.ActivationFunctionType.Exp

T = 2048          # sequence length
DM = 1024         # d_model
DS = 256          # per-core d_model slice (4 heads x 64)
HD = 64           # head dim
NH = 4            # heads per core
KT128 = 16        # k tiles of 128 over T
QB = 512          # q block width
NJ = T // QB      # 4 q blocks
NCORES = 8

_CACHE = {}


def _build():
    nc = bacc.Bacc("TRN2", target_bir_lowering=False, debug=False,
                   enable_asserts=False, num_devices=NCORES)
    dram = {}
    for name, shape, dt in [
        ("xt", [NJ, 128, 8, 512], BF16),  # x[b]^T, t-slice-major chunks
        ("wqt", [DM, DS], BF16),      # wq.T[:, slice] * 0.125
        ("wkt", [DM, DS], BF16),
        ("wvt", [DM, DS], BF16),
        ("wot", [DS, DM], BF16),      # wo[:, slice].T
        ("bqc", [128, 2], F32),       # bq*0.125 as [128, m] columns
        ("bkc", [128, 2], F32),
        ("tri", [128, 128], BF16),    # upper-tri (incl diag) ones
    ]:
        dram[name] = nc.dram_tensor(name, shape, dt, kind="ExternalInput").ap()
    p_out = nc.dram_tensor("p_out", [T, DM], BF16, kind="ExternalOutput").ap()

    with tile.TileContext(nc) as tc:
        with tc.tile_pool(name="persist", bufs=1) as pp, \
             tc.tile_pool(name="epool", bufs=4) as ep, \
             tc.tile_pool(name="outp", bufs=2) as op, \
             tc.tile_pool(name="bcp", bufs=2) as bp, \
             tc.tile_pool(name="misc_ps", bufs=2, space="PSUM") as mp, \
             tc.tile_pool(name="st_ps", bufs=2, space="PSUM") as sp, \
             tc.tile_pool(name="ot_ps", bufs=1, space="PSUM") as tp:

            # ---- persistent SBUF ----
            xt = pp.tile([128, NJ, 8, 512], BF16, name="xt")  # [p, ts, kt8, c]
            wqt = pp.tile([128, 8, DS], BF16, name="wqt")
            wkt = pp.tile([128, 8, DS], BF16, name="wkt")
            wvt = pp.tile([128, 8, DS], BF16, name="wvt")
            wot = pp.tile([128, 2, DM], BF16, name="wot")
            qt = pp.tile([128, 2, T], BF16, name="qt")        # [p, dq-tile, t]
            kt = pp.tile([128, 2, T], BF16, name="kt")
            vaug = pp.tile([128, KT128, 512], BF16, name="vaug")
            ot_all = pp.tile([128, 2, T], BF16, name="ot_all")
            ones_row = pp.tile([1, 512], BF16, name="ones_row")
            bqc = pp.tile([128, 2], F32, name="bqc")
            bkc = pp.tile([128, 2], F32, name="bkc")
            tri = pp.tile([128, 128], BF16, name="tri")

            # ones_row feeds the PE warmup; DVE is otherwise idle at start.
            nc.vector.memset(ones_row, 1.0)
            # PE warmup: dummy matmuls on scratch during the input DMA wait so
            # HAM is at full clock when real matmuls start (results unread)
            warm = mp.tile([128, 512], F32, name="warm", tag="mp")
            for _ in range(9):
                nc.tensor.matmul(warm, lhsT=ones_row[0:1, 0:128],
                                 rhs=ones_row[0:1, :], start=True, stop=True)

            # Input DMA. A single queue sustains only ~100GB/s, so the head
            # load is split in priority order across the three trigger-capable
            # engines (sync/scalar/gpsimd), in chunks small enough that the
            # proj(0) contraction can start while the rest streams in.
            def w_dram(nm):
                return dram[nm].rearrange("(kt p) d -> p kt d", p=128)

            wq_d, wk_d, wv_d, wo_d = (w_dram(n) for n in
                                      ("wqt", "wkt", "wvt", "wot"))
            nc.sync.dma_start(out=xt[:, 0], in_=dram["xt"][0])
            nc.sync.dma_start(out=wvt, in_=wv_d)
            nc.sync.dma_start(
                out=xt[:, 1:4],
                in_=dram["xt"][1:4].rearrange("ts p kt c -> p ts kt c"))
            nc.gpsimd.dma_start(out=wqt, in_=wq_d)
            nc.gpsimd.dma_start(out=wot, in_=wo_d)
            nc.scalar.dma_start(out=wkt, in_=wk_d)
            nc.scalar.dma_start(out=bqc, in_=dram["bqc"])
            nc.scalar.dma_start(out=bkc, in_=dram["bkc"])
            nc.scalar.dma_start(out=tri, in_=dram["tri"])
            # V_aug head block h: cols [128h, 128h+64) ones, [128h+64, +128) V.
            # After the DMA triggers so they don't delay the weight loads.
            for h in range(NH):
                nc.gpsimd.memset(vaug[:, :, 128 * h:128 * h + HD], 1.0)

            # ---- emission helpers ----
            def proj_qk_m(w_sb, b_c, dst, ts, m, chunk=99):
                """One [128,512] psum tile of the Q/K projection; yields every
                `chunk` contraction matmuls."""
                t0 = ts * 512
                ps = mp.tile([128, 512], F32, name="proj_ps", tag="mp")
                for k in range(8):
                    nc.tensor.matmul(
                        ps, lhsT=w_sb[:, k, m * 128:(m + 1) * 128],
                        rhs=xt[:, ts, k, :],
                        start=(k == 0), stop=(k == 7))
                    if k % chunk == chunk - 1 and k < 7:
                        yield
                nc.vector.tensor_scalar_add(
                    dst[:, m, t0:t0 + 512], ps, b_c[:, m:m + 1])
                yield

            def proj_v_tt(g):
                """V projection for t-subtile g (128 rows): 8 matmuls + the
                scatter into vaug."""
                ps = mp.tile([128, 256], F32, name="v_ps", tag="mp")
                for k in range(8):
                    nc.tensor.matmul(
                        ps, lhsT=xt[:, g // 4, k, (g % 4) * 128:(g % 4 + 1) * 128],
                        rhs=wvt[:, k, :], start=(k == 0), stop=(k == 7))
                    if k == 3:
                        yield
                # scatter into vaug: head h -> cols [128h+64, 128h+128)
                nc.vector.tensor_copy(
                    vaug[:, g, :].rearrange("p (h c) -> p h c", h=NH)[:, :, HD:],
                    ps.rearrange("p (h c) -> p h c", h=NH))
                yield

            def gen_proj_qk(ts):
                for w_sb, b_c, dst in ((wqt, bqc, qt), (wkt, bkc, kt)):
                    for m in range(2):
                        yield from proj_qk_m(w_sb, b_c, dst, ts, m, chunk=2)

            def gen_proj_v(ts):
                for tt in range(4):
                    yield from proj_v_tt(4 * ts + tt)

            def gen_wo(j):
                q0 = j * QB
                last = j == NJ - 1
                ob = op.tile([128, 4, DM], BF16, name="ob", tag="ob")
                for qq in range(4):
                    row = q0 + qq * 128
                    for n in range(2):
                        ps = mp.tile([128, 512], F32, name="wo_ps", tag="mp")
                        for kk in range(2):
                            nc.tensor.matmul(
                                ps, lhsT=ot_all[:, kk, row:row + 128],
                                rhs=wot[:, kk, n * 512:(n + 1) * 512],
                                start=(kk == 0), stop=(kk == 1))
                        nc.vector.tensor_copy(ob[:, qq, n * 512:(n + 1) * 512],
                                              ps)
                        yield
                    if last:
                        # tail round: stream the output per 128-row chunk so
                        # the final DMA only trails the last copy
                        dma_eng = nc.sync if qq % 2 == 0 else nc.gpsimd
                        dma_eng.dma_start(
                            out=p_out[row:row + 128, :], in_=ob[:, qq, :])
                if not last:
                    dma_eng = nc.sync if j % 2 == 0 else nc.gpsimd
                    dma_eng.dma_start(
                        out=p_out[q0:q0 + 512, :].rearrange(
                            "(q p) d -> p q d", p=128),
                        in_=ob)
                yield

            # Global filler queue: (key, generator), drained strictly FIFO.
            fillers = deque()

            def fill(n):
                while n > 0 and fillers:
                    try:
                        next(fillers[0][1])
                        n -= 1
                    except StopIteration:
                        fillers.popleft()

            def force(key):
                """Drain fillers until generator `key` has been exhausted."""
                while any(k == key for k, _ in fillers):
                    try:
                        next(fillers[0][1])
                    except StopIteration:
                        fillers.popleft()

            def attention(j, fill_fn):
                q0 = j * QB
                nk = 4 * (j + 1)           # k-tiles of 128
                for H in range(2):          # head pair (2H, 2H+1)
                    ot = [tp.tile([128, QB], F32, name=f"ot{hp}", tag=f"ot{hp}")
                          for hp in range(2)]

                    def pv(ktile, c0, e_t):
                        for hp in range(2):
                            h = 2 * H + hp
                            nc.tensor.matmul(
                                ot[hp][:, c0:QB],
                                lhsT=vaug[:, ktile, 128 * h:128 * (h + 1)],
                                rhs=e_t[:, hp, c0:QB],
                                start=(ktile == 0), stop=(ktile == nk - 1))

                    pend = None     # PV lags one ktile so it never waits exp
                    for ktile in range(nk):
                        s = ktile - 4 * j       # >=0 on diag block
                        c0 = 128 * s if s >= 0 else 0
                        st = sp.tile([128, 2, 512], F32, name="st", tag="st")
                        e_t = ep.tile([128, 2, QB], BF16, name="e", tag="e")
                        for hp in range(2):
                            h = 2 * H + hp
                            r0 = (HD * h) % 128
                            mi = (HD * h) // 128
                            nc.tensor.matmul(
                                st[:, hp, c0:512],
                                lhsT=kt[r0:r0 + HD, mi,
                                        ktile * 128:(ktile + 1) * 128],
                                rhs=qt[r0:r0 + HD, mi, q0 + c0:q0 + QB],
                                start=True, stop=True)
                        # diag tiles: only cols [c0:512) are computed/read --
                        # skip the stale region in the exp too
                        nc.scalar.activation(
                            out=e_t[:, :, c0:512], in_=st[:, :, c0:512],
                            func=EXP, scale=1.0)
                        if s >= 0:
                            # mask both heads' diag tile in one strided op;
                            # gpsimd keeps it off the busier DVE queue
                            dg = e_t[:, :, c0:c0 + 128]
                            trb = bass.AP(
                                tensor=tri.tensor, offset=tri.offset,
                                ap=[tri.ap[0], [0, 2], [1, 128]])
                            nc.gpsimd.tensor_mul(dg, dg, trb)
                        fill_fn(j, H, ktile)
                        if pend is not None:
                            pv(*pend)
                        pend = (ktile, c0, e_t)
                    pv(*pend)
                    for hp in range(2):
                        h = 2 * H + hp
                        rec = bp.tile([64, QB], F32, name="rec", tag="rec")
                        nc.vector.reciprocal_approx_fast(rec, ot[hp][0:64, :])
                        r0 = (HD * h) % 128
                        mi = (HD * h) // 128
                        nc.vector.tensor_mul(
                            ot_all[r0:r0 + HD, mi, q0:q0 + QB],
                            ot[hp][64:128, :], rec)

            # ---- schedule ----
            # Round 0 prefix: Q m0, K m0, V g=0 -> attention(0) H0 can start;
            # the rest of proj(0) rides along as round-0 filler.
            for _ in proj_qk_m(wqt, bqc, qt, 0, 0):
                pass
            for _ in proj_qk_m(wkt, bkc, kt, 0, 0):
                pass
            for _ in proj_v_tt(0):
                pass
            local = deque()
            for g in (proj_v_tt(1), proj_v_tt(2), proj_v_tt(3),
                      proj_qk_m(wqt, bqc, qt, 0, 1, chunk=4),
                      proj_qk_m(wkt, bkc, kt, 0, 1, chunk=4)):
                local.append(("r0", g))

            def fill_r0(j, H, ktile):
                n = 3
                while n > 0 and local:
                    try:
                        next(local[0][1])
                        n -= 1
                    except StopIteration:
                        local.popleft()
                if not local:
                    fill(1)

            for ts in range(1, NJ):
                fillers.append((("qk", ts), gen_proj_qk(ts)))
                fillers.append((("v", ts), gen_proj_v(ts)))

            def fill_main(j, H, ktile):
                if H == 0 and ktile == 4 * j:
                    force(("v", j))   # vaug t-slice j gate (usually a no-op)
                fill(1)

            attention(0, fill_r0)
            while local:
                fill_r0(0, 0, 0)
            for j in range(1, NJ):
                force(("qk", j))
                if j == NJ - 1:
                    # wo is the only filler left whose deadline is the kernel
                    # end; spend it on the most ACT-bound round.
                    for jj in range(NJ - 1):
                        fillers.append((("wo", jj), gen_wo(jj)))
                attention(j, fill_main)
            while fillers:
                fill(64)
            for _ in gen_wo(NJ - 1):
                pass
    nc.compile()
    return nc


def _prep_inputs(x, wq, bq, wk, bk, wv, wo):
    bf = ml_dtypes.bfloat16
    scale = np.float32(1.0 / np.sqrt(HD))
    tri = np.triu(np.ones((128, 128), np.float32)).astype(bf)
    in_maps = []
    for c in range(NCORES):
        b, g = c // 4, c % 4
        sl = slice(DS * g, DS * (g + 1))
        xts = np.ascontiguousarray(
            x[b].T.astype(bf).reshape(8, 128, 4, 512).transpose(2, 1, 0, 3))
        in_maps.append({
            "xt": xts,
            "wqt": np.ascontiguousarray(wq.T[:, sl] * scale).astype(bf),
            "wkt": np.ascontiguousarray(wk.T[:, sl]).astype(bf),
            "wvt": np.ascontiguousarray(wv.T[:, sl]).astype(bf),
            "wot": np.ascontiguousarray(wo[:, sl].T).astype(bf),
            "bqc": np.ascontiguousarray(
                (bq[sl] * scale).reshape(2, 128).T).astype(np.float32),
            "bkc": np.ascontiguousarray(
                bk[sl].reshape(2, 128).T).astype(np.float32),
            "tri": tri,
        })
    return in_maps


TRACE = False
TRACE_DIR = None
LAST_RESULT = None


def kernel(x, wq, bq, wk, bk, wv, bv, wo, bo):
    global LAST_RESULT
    x, wq, bq, wk, bk, wv, bv, wo, bo = [
        np.asarray(a, np.float32)
        for a in (x, wq, bq, wk, bk, wv, bv, wo, bo)]
    if "nc" not in _CACHE:
        _CACHE["nc"] = _build()
    nc = _CACHE["nc"]
    in_maps = _prep_inputs(x, wq, bq, wk, bk, wv, wo)
    res = bass_utils.run_bass_kernel_spmd(
        nc, in_maps, core_ids=list(range(NCORES)), trace=TRACE,
        tmpdir=TRACE_DIR)
    LAST_RESULT = res
    # bv commutes through softmax (rows sum to 1): out += bv @ wo^T + bo.
    const_row = (bv.astype(np.float64) @ wo.T.astype(np.float64) +
                 bo.astype(np.float64)).astype(np.float32)
    out = np.empty((2, T, DM), np.float32)
    for b in range(2):
        acc = res.results[4 * b]["p_out"].astype(np.float32)
        for g in range(1, 4):
            acc = acc + res.results[4 * b + g]["p_out"].astype(np.float32)
        out[b] = acc + const_row
    return out


# revision 15
# speedup vs baseline: 1.0552x; 1.0051x over previous
"""Multi-head self-attention (B=2, T=2048, d_model=1024, 16 heads, causal)
on 8 trn2 NeuronCores.

Sharding: core c -> batch b=c//4, head-group g=c%4 (4 heads, d_model slice
of 256). Each core computes its heads' attention and a partial wo
projection [2048, 1024] (bf16); host sums the 4 partials per batch and
adds bo + bv @ wo^T (the V-bias commutes through softmax since rows sum
to 1, so it is a host-side constant).

Per-core pipeline (all matmul inputs bf16, fp32 PSUM accumulation):
  QT[dq,t] = (wq_s @ x^T)*0.125 + bq*0.125   (scale folded into weights)
  KT[dk,t] = wk_s @ x^T + bk
  V[t,dv]  = x @ wv_s^T
  per head pair, per q-block j (512 wide):
    S^T[k,q] = K_h @ Q_h^T   (K=64 contraction, heads packed at partition
                              bases 0/64 -> concurrent row-group matmuls)
    E = exp(S^T)             (ACT, one call per (head-pair, ktile))
    diag tiles: E *= upper-tri mask
    O^T_aug = V_aug^T @ E    (V_aug = [64 ones cols | V_h cols] so PSUM rows
                              0-63 = replicated rowsums, rows 64-127 = O^T)
    OT_all = O^T * recip(rowsum)  (DVE reciprocal + tensor_tensor mul)
  P = OT_all^T @ wo_s^T      (partial output, bf16 out)

Scheduling: the per-ktile chain S(PE) -> exp(ACT) -> PV(PE) leaves PE
under-filled during attention (ACT needs ~1.15us per ktile vs ~0.64us of
attention PE work).  Projection and wo matmuls for neighboring q-blocks
are emitted as generator-based filler chunks (~0.4us of PE work per
chunk) interleaved between attention ktile steps; wo work is held back
to round 3, the most ACT-bound stretch.  x is staged t-slice-major
(both DRAM and SBUF) so each input DMA moves 8KB-contiguous lines per
partition at full rate and projections of t-slice 0 start ~10us in.
"""
import sys
sys.path.insert(0, "/opt/trn_rl_repo")
from collections import deque

import numpy as np
import ml_dtypes

import concourse.bass as bass
import concourse.bacc as bacc
import concourse.tile as tile
import concourse.mybir as mybir
from concourse import bass_utils

BF16 = mybir.dt.bfloat16
F32 = mybir.dt.float32
EXP = mybir.ActivationFunctionType.Exp

T = 2048          # sequence length
DM = 1024         # d_model
DS = 256          # per-core d_model slice (4 heads x 64)
HD = 64           # head dim
NH = 4            # heads per core
KT128 = 16        # k tiles of 128 over T
QB = 512          # q block width
NJ = T // QB      # 4 q blocks
NCORES = 8

_CACHE = {}


def _build():
    nc = bacc.Bacc("TRN2", target_bir_lowering=False, debug=False,
                   enable_asserts=False, num_devices=NCORES)
    dram = {}
    for name, shape, dt in [
        ("xt", [NJ, 128, 8, 512], BF16),  # x[b]^T, t-slice-major chunks
        ("wqt", [DM, DS], BF16),      # wq.T[:, slice] * 0.125
        ("wkt", [DM, DS], BF16),
        ("wvt", [DM, DS], BF16),
        ("wot", [DS, DM], BF16),      # wo[:, slice].T
        ("bqc", [128, 2], F32),       # bq*0.125 as [128, m] columns
        ("bkc", [128, 2], F32),
        ("tri", [128, 128], BF16),    # upper-tri (incl diag) ones
    ]:
        dram[name] = nc.dram_tensor(name, shape, dt, kind="ExternalInput").ap()
    p_out = nc.dram_tensor("p_out", [T, DM], BF16, kind="ExternalOutput").ap()

    with tile.TileContext(nc) as tc:
        with tc.tile_pool(name="persist", bufs=1) as pp, \
             tc.tile_pool(name="epool", bufs=4) as ep, \
             tc.tile_pool(name="outp", bufs=2) as op, \
             tc.tile_pool(name="bcp", bufs=2) as bp, \
             tc.tile_pool(name="misc_ps", bufs=2, space="PSUM") as mp, \
             tc.tile_pool(name="st_ps", bufs=2, space="PSUM") as sp, \
             tc.tile_pool(name="ot_ps", bufs=1, space="PSUM") as tp:

            # ---- persistent SBUF ----
            xt = pp.tile([128, NJ, 8, 512], BF16, name="xt")  # [p, ts, kt8, c]
            wqt = pp.tile([128, 8, DS], BF16, name="wqt")
            wkt = pp.tile([128, 8, DS], BF16, name="wkt")
            wvt = pp.tile([128, 8, DS], BF16, name="wvt")
            wot = pp.tile([128, 2, DM], BF16, name="wot")
            qt = pp.tile([128, 2, T], BF16, name="qt")        # [p, dq-tile, t]
            kt = pp.tile([128, 2, T], BF16, name="kt")
            vaug = pp.tile([128, KT128, 512], BF16, name="vaug")
            ot_all = pp.tile([128, 2, T], BF16, name="ot_all")
            ones_row = pp.tile([1, 512], BF16, name="ones_row")
            bqc = pp.tile([128, 2], F32, name="bqc")
            bkc = pp.tile([128, 2], F32, name="bkc")
            tri = pp.tile([128, 128], BF16, name="tri")

            # ones_row feeds the PE warmup; DVE is otherwise idle at start.
            nc.vector.memset(ones_row, 1.0)
            # PE warmup: dummy matmuls on scratch during the input DMA wait so
            # HAM is at full clock when real matmuls start (results unread)
            warm = mp.tile([128, 512], F32, name="warm", tag="mp")
            for _ in range(9):
                nc.tensor.matmul(warm, lhsT=ones_row[0:1, 0:128],
                                 rhs=ones_row[0:1, :], start=True, stop=True)

            # Input DMA. A single queue sustains only ~100GB/s, so the head
            # load is split in priority order across the three trigger-capable
            # engines (sync/scalar/gpsimd), in chunks small enough that the
            # proj(0) contraction can start while the rest streams in.
            def w_dram(nm):
                return dram[nm].rearrange("(kt p) d -> p kt d", p=128)

            wq_d, wk_d, wv_d, wo_d = (w_dram(n) for n in
                                      ("wqt", "wkt", "wvt", "wot"))
            # x0 in two chunks: proj(0) k=0..3 matmuls start off the first
            # chunk (subtile deps) while the second still streams
            nc.sync.dma_start(out=xt[:, 0, 0:4], in_=dram["xt"][0, :, 0:4])
            nc.sync.dma_start(out=xt[:, 0, 4:8], in_=dram["xt"][0, :, 4:8])
            nc.sync.dma_start(out=wvt, in_=wv_d)
            nc.sync.dma_start(
                out=xt[:, 1:4],
                in_=dram["xt"][1:4].rearrange("ts p kt c -> p ts kt c"))
            nc.gpsimd.dma_start(out=wqt, in_=wq_d)
            nc.gpsimd.dma_start(out=wot, in_=wo_d)
            nc.scalar.dma_start(out=wkt, in_=wk_d)
            nc.scalar.dma_start(out=bqc, in_=dram["bqc"])
            nc.scalar.dma_start(out=bkc, in_=dram["bkc"])
            nc.scalar.dma_start(out=tri, in_=dram["tri"])
            # V_aug head block h: cols [128h, 128h+64) ones, [128h+64, +128) V.
            # After the DMA triggers so they don't delay the weight loads.
            for h in range(NH):
                nc.gpsimd.memset(vaug[:, :, 128 * h:128 * h + HD], 1.0)

            # ---- emission helpers ----
            def proj_qk_m(w_sb, b_c, dst, ts, m, chunk=99):
                """One [128,512] psum tile of the Q/K projection; yields every
                `chunk` contraction matmuls."""
                t0 = ts * 512
                ps = mp.tile([128, 512], F32, name="proj_ps", tag="mp")
                for k in range(8):
                    nc.tensor.matmul(
                        ps, lhsT=w_sb[:, k, m * 128:(m + 1) * 128],
                        rhs=xt[:, ts, k, :],
                        start=(k == 0), stop=(k == 7))
                    if k % chunk == chunk - 1 and k < 7:
                        yield
                if ts < 3:
                    # ACT has slack in the early rounds and reads PSUM fast;
                    # this also frees the psum slot sooner than the busier DVE
                    nc.scalar.activation(
                        out=dst[:, m, t0:t0 + 512], in_=ps,
                        func=mybir.ActivationFunctionType.Identity,
                        bias=b_c[:, m:m + 1])
                else:
                    nc.vector.tensor_scalar_add(
                        dst[:, m, t0:t0 + 512], ps, b_c[:, m:m + 1])
                yield

            def proj_v_tt(g):
                """V projection for t-subtile g (128 rows): 8 matmuls + the
                scatter into vaug."""
                ps = mp.tile([128, 256], F32, name="v_ps", tag="mp")
                for k in range(8):
                    nc.tensor.matmul(
                        ps, lhsT=xt[:, g // 4, k, (g % 4) * 128:(g % 4 + 1) * 128],
                        rhs=wvt[:, k, :], start=(k == 0), stop=(k == 7))
                    if k == 3:
                        yield
                # scatter into vaug: head h -> cols [128h+64, 128h+128)
                nc.vector.tensor_copy(
                    vaug[:, g, :].rearrange("p (h c) -> p h c", h=NH)[:, :, HD:],
                    ps.rearrange("p (h c) -> p h c", h=NH))
                yield

            def gen_proj_qk(ts):
                for w_sb, b_c, dst in ((wqt, bqc, qt), (wkt, bkc, kt)):
                    for m in range(2):
                        yield from proj_qk_m(w_sb, b_c, dst, ts, m, chunk=2)

            def gen_proj_v(ts):
                for tt in range(4):
                    yield from proj_v_tt(4 * ts + tt)

            def gen_wo(j):
                q0 = j * QB
                last = j == NJ - 1
                ob = op.tile([128, 4, DM], BF16, name="ob", tag="ob")
                for qq in range(4):
                    row = q0 + qq * 128
                    for n in range(2):
                        ps = mp.tile([128, 512], F32, name="wo_ps", tag="mp")
                        for kk in range(2):
                            nc.tensor.matmul(
                                ps, lhsT=ot_all[:, kk, row:row + 128],
                                rhs=wot[:, kk, n * 512:(n + 1) * 512],
                                start=(kk == 0), stop=(kk == 1))
                        nc.vector.tensor_copy(ob[:, qq, n * 512:(n + 1) * 512],
                                              ps)
                        yield
                    if last:
                        # tail round: stream the output per 128-row chunk so
                        # the final DMA only trails the last copy
                        dma_eng = nc.sync if qq % 2 == 0 else nc.gpsimd
                        dma_eng.dma_start(
                            out=p_out[row:row + 128, :], in_=ob[:, qq, :])
                if not last:
                    dma_eng = nc.sync if j % 2 == 0 else nc.gpsimd
                    dma_eng.dma_start(
                        out=p_out[q0:q0 + 512, :].rearrange(
                            "(q p) d -> p q d", p=128),
                        in_=ob)
                yield

            # Global filler queue: (key, generator), drained strictly FIFO.
            fillers = deque()

            def fill(n):
                while n > 0 and fillers:
                    try:
                        next(fillers[0][1])
                        n -= 1
                    except StopIteration:
                        fillers.popleft()

            def force(key):
                """Drain fillers until generator `key` has been exhausted."""
                while any(k == key for k, _ in fillers):
                    try:
                        next(fillers[0][1])
                    except StopIteration:
                        fillers.popleft()

            def attention(j, fill_fn):
                q0 = j * QB
                nk = 4 * (j + 1)           # k-tiles of 128
                for H in range(2):          # head pair (2H, 2H+1)
                    ot = [tp.tile([128, QB], F32, name=f"ot{hp}", tag=f"ot{hp}")
                          for hp in range(2)]

                    def pv(ktile, c0, e_t):
                        for hp in range(2):
                            h = 2 * H + hp
                            nc.tensor.matmul(
                                ot[hp][:, c0:QB],
                                lhsT=vaug[:, ktile, 128 * h:128 * (h + 1)],
                                rhs=e_t[:, hp, c0:QB],
                                start=(ktile == 0), stop=(ktile == nk - 1))

                    pend = None     # PV lags one ktile so it never waits exp
                    for ktile in range(nk):
                        s = ktile - 4 * j       # >=0 on diag block
                        c0 = 128 * s if s >= 0 else 0
                        st = sp.tile([128, 2, 512], F32, name="st", tag="st")
                        e_t = ep.tile([128, 2, QB], BF16, name="e", tag="e")
                        for hp in range(2):
                            h = 2 * H + hp
                            r0 = (HD * h) % 128
                            mi = (HD * h) // 128
                            nc.tensor.matmul(
                                st[:, hp, c0:512],
                                lhsT=kt[r0:r0 + HD, mi,
                                        ktile * 128:(ktile + 1) * 128],
                                rhs=qt[r0:r0 + HD, mi, q0 + c0:q0 + QB],
                                start=True, stop=True)
                        # diag tiles: only cols [c0:512) are computed/read --
                        # skip the stale region in the exp too
                        nc.scalar.activation(
                            out=e_t[:, :, c0:512], in_=st[:, :, c0:512],
                            func=EXP, scale=1.0)
                        if s >= 0:
                            # mask both heads' diag tile in one strided op;
                            # gpsimd keeps it off the busier DVE queue
                            dg = e_t[:, :, c0:c0 + 128]
                            trb = bass.AP(
                                tensor=tri.tensor, offset=tri.offset,
                                ap=[tri.ap[0], [0, 2], [1, 128]])
                            nc.gpsimd.tensor_mul(dg, dg, trb)
                        fill_fn(j, H, ktile)
                        if pend is not None:
                            pv(*pend)
                        pend = (ktile, c0, e_t)
                    pv(*pend)
                    for hp in range(2):
                        h = 2 * H + hp
                        rec = bp.tile([64, QB], F32, name="rec", tag="rec")
                        nc.vector.reciprocal_approx_fast(rec, ot[hp][0:64, :])
                        r0 = (HD * h) % 128
                        mi = (HD * h) // 128
                        nc.vector.tensor_mul(
                            ot_all[r0:r0 + HD, mi, q0:q0 + QB],
                            ot[hp][64:128, :], rec)

            # ---- schedule ----
            # Round 0 prefix: Q m0, K m0, V g=0 -> attention(0) H0 can start;
            # the rest of proj(0) rides along as round-0 filler.
            for _ in proj_qk_m(wqt, bqc, qt, 0, 0):
                pass
            for _ in proj_qk_m(wkt, bkc, kt, 0, 0):
                pass
            for _ in proj_v_tt(0):
                pass
            local = deque()
            for g in (proj_v_tt(1), proj_v_tt(2), proj_v_tt(3),
                      proj_qk_m(wqt, bqc, qt, 0, 1, chunk=4),
                      proj_qk_m(wkt, bkc, kt, 0, 1, chunk=4)):
                local.append(("r0", g))

            def fill_r0(j, H, ktile):
                n = 3
                while n > 0 and local:
                    try:
                        next(local[0][1])
                        n -= 1
                    except StopIteration:
                        local.popleft()
                if not local:
                    fill(1)

            for ts in range(1, NJ):
                fillers.append((("qk", ts), gen_proj_qk(ts)))
                fillers.append((("v", ts), gen_proj_v(ts)))

            def fill_main(j, H, ktile):
                if H == 0 and ktile == 4 * j:
                    force(("v", j))   # vaug t-slice j gate (usually a no-op)
                fill(1)

            attention(0, fill_r0)
            while local:
                fill_r0(0, 0, 0)
            for j in range(1, NJ):
                force(("qk", j))
                if j == NJ - 1:
                    # wo is the only filler left whose deadline is the kernel
                    # end; spend it on the most ACT-bound round.
                    for jj in range(NJ - 1):
                        fillers.append((("wo", jj), gen_wo(jj)))
                attention(j, fill_main)
            while fillers:
                fill(64)
            for _ in gen_wo(NJ - 1):
                pass
    nc.compile()
    return nc


def _prep_inputs(x, wq, bq, wk, bk, wv, wo):
    bf = ml_dtypes.bfloat16
    scale = np.float32(1.0 / np.sqrt(HD))
    tri = np.triu(np.ones((128, 128), np.float32)).astype(bf)
    in_maps = []
    for c in range(NCORES):
        b, g = c // 4, c % 4
        sl = slice(DS * g, DS * (g + 1))
        xts = np.ascontiguousarray(
            x[b].T.astype(bf).reshape(8, 128, 4, 512).transpose(2, 1, 0, 3))
        in_maps.append({
            "xt": xts,
            "wqt": np.ascontiguousarray(wq.T[:, sl] * scale).astype(bf),
            "wkt": np.ascontiguousarray(wk.T[:, sl]).astype(bf),
            "wvt": np.ascontiguousarray(wv.T[:, sl]).astype(bf),
            "wot": np.ascontiguousarray(wo[:, sl].T).astype(bf),
            "bqc": np.ascontiguousarray(
                (bq[sl] * scale).reshape(2, 128).T).astype(np.float32),
            "bkc": np.ascontiguousarray(
                bk[sl].reshape(2, 128).T).astype(np.float32),
            "tri": tri,
        })
    return in_maps


TRACE = False
TRACE_DIR = None
LAST_RESULT = None


def kernel(x, wq, bq, wk, bk, wv, bv, wo, bo):
    global LAST_RESULT
    x, wq, bq, wk, bk, wv, bv, wo, bo = [
        np.asarray(a, np.float32)
        for a in (x, wq, bq, wk, bk, wv, bv, wo, bo)]
    if "nc" not in _CACHE:
        _CACHE["nc"] = _build()
    nc = _CACHE["nc"]
    in_maps = _prep_inputs(x, wq, bq, wk, bk, wv, wo)
    res = bass_utils.run_bass_kernel_spmd(
        nc, in_maps, core_ids=list(range(NCORES)), trace=TRACE,
        tmpdir=TRACE_DIR)
    LAST_RESULT = res
    # bv commutes through softmax (rows sum to 1): out += bv @ wo^T + bo.
    const_row = (bv.astype(np.float64) @ wo.T.astype(np.float64) +
                 bo.astype(np.float64)).astype(np.float32)
    out = np.empty((2, T, DM), np.float32)
    for b in range(2):
        acc = res.results[4 * b]["p_out"].astype(np.float32)
        for g in range(1, 4):
            acc = acc + res.results[4 * b + g]["p_out"].astype(np.float32)
        out[b] = acc + const_row
    return out
